# revision 5
# baseline (speedup 1.0000x reference)
"""Trainium2 Bass kernel for nn_ABSEncoder (8-core data-parallel over batch).

reference:
    mask = (x == 0)                                   # [B, SRC]
    xe  = F_emb[x]                                    # [B, SRC, D]
    yce = G_emb[yc].reshape(B, SEQ, CTX*D)            # [B, SEQ, CTX*D]
    py  = yce @ P_w + P_b                             # [B, SEQ, D]
    a   = einsum('bxd,bsd->bxs', xe, py) + mask*-1e9
    a   = softmax(a.transpose(0,2,1), axis=-1)        # [B, SEQ, SRC]
    out = einsum('bsx,bxd->bsd', a, xe)               # [B, SEQ, D]
    return (out, a)

Strategy: batch is sharded 4 per core; embedding tables (converted to bf16 on
host) are replicated. Gathers use gpsimd indirect DMA (one per tensor per
batch). All contractions run on the PE in bf16 with f32 PSUM accumulation;
operand transposes (d-major / x-major layouts) are PE identity-matmul
transposes. The source-padding mask is folded into the logits matmul as a
K=1 rank-1 update (-1e9 outer (x==0)). Softmax normalization is folded into
the output copies as a per-partition scale.
"""

import numpy as np
import ml_dtypes

B, SRC = 32, 1024
SEQ, CTX, D, V = 128, 5, 512, 32000
N_CORES = 8
BPC = B // N_CORES  # batches per core
P = 128
XCH = SRC // P   # 8 x-chunks of 128 tokens
DCH = D // P     # 4 d-chunks
KCH = CTX * DCH  # 20 k-chunks of the dense contraction

_compiled = {}


def _build():
    import concourse.bass as bass
    import concourse.tile as tile
    from concourse import bacc, mybir
    from concourse.masks import make_identity

    f32 = mybir.dt.float32
    bf16 = mybir.dt.bfloat16
    i32 = mybir.dt.int32

    nc = bacc.Bacc("TRN2", target_bir_lowering=False, debug=False,
                   num_devices=N_CORES)

    # DRAM parameters (per core)
    F_d = nc.dram_tensor("F", [V, D], bf16, kind="ExternalInput")
    G_d = nc.dram_tensor("G", [V, D], bf16, kind="ExternalInput")
    PW_d = nc.dram_tensor("PW", [CTX * D, D], bf16, kind="ExternalInput")
    PB_d = nc.dram_tensor("PB", [1, D], bf16, kind="ExternalInput")
    XI_d = nc.dram_tensor("XI", [BPC, P, XCH], i32, kind="ExternalInput")
    YI_d = nc.dram_tensor("YI", [BPC, P, CTX], i32, kind="ExternalInput")
    M_d = nc.dram_tensor("M01", [BPC, SRC], bf16, kind="ExternalInput")
    out_d = nc.dram_tensor("out_o", [BPC, P, D], f32, kind="ExternalOutput")
    a_d = nc.dram_tensor("a_o", [BPC, P, SRC], f32, kind="ExternalOutput")
    DBG = __import__("os").environ.get("KDBG") == "1"
    if DBG:
        dbg_py = nc.dram_tensor("dbg_py", [BPC, P, D], f32, kind="ExternalOutput")
        dbg_nm = nc.dram_tensor("dbg_nm", [BPC, P, 4], f32, kind="ExternalOutput")
        dbg_xe = nc.dram_tensor("dbg_xe", [BPC, P, XCH, D], f32, kind="ExternalOutput")

    with tile.TileContext(nc) as tc:
        with (
            tc.tile_pool(name="singles", bufs=1) as singles,
            tc.tile_pool(name="work", bufs=2) as work,
            tc.tile_pool(name="psum_tr", bufs=4, space="PSUM") as psum_tr,
            tc.tile_pool(name="psum_mm", bufs=3, space="PSUM") as psum_mm,
        ):
            # resident weights / constants
            pw_sb = singles.tile([P, KCH, D], bf16)  # P_w k-chunk j rows
            nc.sync.dma_start(
                out=pw_sb[:],
                in_=PW_d.ap().rearrange("(j p) d -> p j d", p=P),
            )
            pb_sb = singles.tile([1, D], bf16)
            nc.sync.dma_start(out=pb_sb[:], in_=PB_d.ap()[:])
            ident = singles.tile([P, P], bf16)
            make_identity(nc, ident[:])
            ident_f = singles.tile([P, P], f32)
            make_identity(nc, ident_f[:])
            ones_r = singles.tile([1, P], bf16)
            nc.gpsimd.memset(ones_r[:], 1.0)
            neg_r = singles.tile([1, P], bf16)
            nc.gpsimd.memset(neg_r[:], -1e9)

            for b in range(BPC):
                # ---- index / mask loads ----
                xi = work.tile([P, XCH], i32)
                nc.sync.dma_start(out=xi[:], in_=XI_d.ap()[b])
                yi = work.tile([P, CTX], i32)
                nc.sync.dma_start(out=yi[:], in_=YI_d.ap()[b])
                m01 = work.tile([1, SRC], bf16)
                nc.sync.dma_start(out=m01[:], in_=M_d.ap()[b:b + 1, :])

                # ---- gathers ----
                xe = work.tile([P, XCH, D], bf16)  # xe[p, j, :] = F[x[b, j*128+p]]
                for j in range(XCH):
                    nc.gpsimd.indirect_dma_start(
                        out=xe[:, j, :], out_offset=None,
                        in_=F_d.ap()[:],
                        in_offset=bass.IndirectOffsetOnAxis(ap=xi[:, j:j + 1], axis=0),
                    )
                yce = work.tile([P, CTX, D], bf16)  # yce[s, c, :] = G[yc[b, 5s+c]]
                for c in range(CTX):
                    nc.gpsimd.indirect_dma_start(
                        out=yce[:, c, :], out_offset=None,
                        in_=G_d.ap()[:],
                        in_offset=bass.IndirectOffsetOnAxis(ap=yi[:, c:c + 1], axis=0),
                    )

                # ---- yce^T: [s, k] -> [k, s] in 20 PE-transposed blocks ----
                yceT = work.tile([P, KCH, P], bf16)
                for g in range(KCH // 4):  # 5 psum groups of 4 blocks
                    ps = psum_tr.tile([P, 4, P], bf16, tag="tr")
                    for q in range(4):
                        j = g * 4 + q
                        c, dj = divmod(j, DCH)
                        nc.tensor.transpose(
                            out=ps[:, q, :],
                            in_=yce[:, c, dj * P:(dj + 1) * P],
                            identity=ident[:],
                        )
                    nc.scalar.copy(out=yceT[:, g * 4:(g + 1) * 4, :], in_=ps[:])

                # ---- py = yce @ P_w + P_b  (PSUM [s, d]) ----
                py_ps = psum_mm.tile([P, D], f32, tag="mm")
                for j in range(KCH):
                    nc.tensor.matmul(
                        out=py_ps[:], lhsT=yceT[:, j, :], rhs=pw_sb[:, j, :],
                        start=(j == 0), stop=False,
                    )
                nc.tensor.matmul(
                    out=py_ps[:], lhsT=ones_r[:], rhs=pb_sb[:],
                    start=False, stop=True,
                )
                py_sb = work.tile([P, D], bf16)
                nc.scalar.copy(out=py_sb[:], in_=py_ps[:])
                if DBG:
                    dpy = work.tile([P, D], f32)
                    nc.vector.tensor_copy(out=dpy[:], in_=py_ps[:])
                    nc.sync.dma_start(out=dbg_py.ap()[b], in_=dpy[:])
                    dxe = work.tile([P, XCH, D], f32)
                    nc.vector.tensor_copy(out=dxe[:], in_=xe[:])
                    nc.sync.dma_start(out=dbg_xe.ap()[b], in_=dxe[:])

                # ---- py^T: [s, d] -> [d, s] ----
                pyT = work.tile([P, DCH, P], bf16)
                psT = psum_tr.tile([P, 4, P], bf16, tag="tr")
                for dj in range(DCH):
                    nc.tensor.transpose(
                        out=psT[:, dj, :],
                        in_=py_sb[:, dj * P:(dj + 1) * P],
                        identity=ident[:],
                    )
                nc.scalar.copy(out=pyT[:], in_=psT[:])

                # ---- xe^T: [x, d] -> [d, x] in 32 blocks ----
                xeT = work.tile([P, DCH, XCH, P], bf16)
                for dj in range(DCH):
                    for h in range(2):  # two groups of 4 x-chunks
                        ps = psum_tr.tile([P, 4, P], bf16, tag="tr")
                        for q in range(4):
                            xj = h * 4 + q
                            nc.tensor.transpose(
                                out=ps[:, q, :],
                                in_=xe[:, xj, dj * P:(dj + 1) * P],
                                identity=ident[:],
                            )
                        nc.vector.tensor_copy(
                            out=xeT[:, dj, h * 4:(h + 1) * 4, :], in_=ps[:])

                # ---- logits a[s, x] = py @ xe^T + (-1e9) * mask01[x] ----
                a_ps = []
                for h in range(2):  # two 512-wide halves of x
                    ps = psum_mm.tile([P, D], f32, tag="mm")
                    for dj in range(DCH):
                        nc.tensor.matmul(
                            out=ps[:], lhsT=pyT[:, dj, :],
                            rhs=xeT[:, dj, h * 4:(h + 1) * 4, :],
                            start=(dj == 0), stop=False,
                        )
                    nc.tensor.matmul(
                        out=ps[:], lhsT=neg_r[:],
                        rhs=m01[:, h * D:(h + 1) * D],
                        start=False, stop=True,
                    )
                    a_ps.append(ps)

                # ---- softmax over x (free axis) ----
                mx2 = work.tile([P, 2], f32)
                for h in range(2):
                    nc.vector.tensor_reduce(
                        out=mx2[:, h:h + 1], in_=a_ps[h][:],
                        axis=mybir.AxisListType.X, op=mybir.AluOpType.max,
                    )
                negmax = work.tile([P, 1], f32)
                nc.vector.tensor_reduce(
                    out=negmax[:], in_=mx2[:],
                    axis=mybir.AxisListType.X, op=mybir.AluOpType.max,
                    negate=True,
                )
                p_sb = work.tile([P, SRC], f32)
                zacc = work.tile([P, 2], f32)
                for h in range(2):
                    nc.scalar.activation(
                        out=p_sb[:, h * D:(h + 1) * D], in_=a_ps[h][:],
                        func=mybir.ActivationFunctionType.Exp,
                        bias=negmax[:], scale=1.0,
                        accum_out=zacc[:, h:h + 1],
                    )
                rz = work.tile([P, 1], f32)
                nc.vector.tensor_reduce(
                    out=rz[:], in_=zacc[:],
                    axis=mybir.AxisListType.X, op=mybir.AluOpType.add,
                )
                nc.vector.reciprocal(out=rz[:], in_=rz[:])
                if DBG:
                    dnm = work.tile([P, 4], f32)
                    nc.vector.tensor_copy(out=dnm[:, 0:1], in_=negmax[:])
                    nc.vector.tensor_copy(out=dnm[:, 1:3], in_=zacc[:])
                    nc.vector.tensor_copy(out=dnm[:, 3:4], in_=rz[:])
                    nc.sync.dma_start(out=dbg_nm.ap()[b], in_=dnm[:])

                # a output = p * (1/Z)
                a_sb = work.tile([P, SRC], f32)
                nc.scalar.mul(a_sb[:], p_sb[:], rz[:])
                nc.sync.dma_start(out=a_d.ap()[b], in_=a_sb[:])

                # ---- p^T: [s, x] -> [x, s] in 8 blocks ----
                pT = work.tile([P, XCH, P], bf16)
                for h in range(2):
                    ps = psum_tr.tile([P, 4, P], f32, tag="tr")
                    for q in range(4):
                        xj = h * 4 + q
                        nc.tensor.transpose(
                            out=ps[:, q, :],
                            in_=p_sb[:, xj * P:(xj + 1) * P],
                            identity=ident_f[:],
                        )
                    nc.scalar.copy(out=pT[:, h * 4:(h + 1) * 4, :], in_=ps[:])

                # ---- out[s, d] = (p @ xe) * (1/Z) ----
                o_ps = psum_mm.tile([P, D], f32, tag="mm")
                for xj in range(XCH):
                    nc.tensor.matmul(
                        out=o_ps[:], lhsT=pT[:, xj, :], rhs=xe[:, xj, :],
                        start=(xj == 0), stop=(xj == XCH - 1),
                    )
                o_sb = work.tile([P, D], f32)
                nc.scalar.mul(o_sb[:], o_ps[:], rz[:])
                nc.sync.dma_start(out=out_d.ap()[b], in_=o_sb[:])

    nc.compile()
    return nc


def _get_nc():
    if "nc" not in _compiled:
        _compiled["nc"] = _build()
    return _compiled["nc"]


def kernel(x, yc, F_emb, G_emb, P_w, P_b):
    from concourse.bass_utils import run_bass_kernel_spmd

    nc = _get_nc()

    x = np.asarray(x).astype(np.int32)
    yc = np.asarray(yc).astype(np.int32)
    bf = ml_dtypes.bfloat16
    Fb = np.asarray(F_emb, dtype=np.float32).astype(bf)
    Gb = np.asarray(G_emb, dtype=np.float32).astype(bf)
    PWb = np.asarray(P_w, dtype=np.float32).astype(bf)
    PBb = np.asarray(P_b, dtype=np.float32).astype(bf).reshape(1, D)
    m01 = (x == 0).astype(bf)  # [B, SRC]

    # xi[b, p, j] = x[b, j*128 + p];  yi[b, s, c] = yc[b, 5s + c]
    xi = np.ascontiguousarray(x.reshape(B, XCH, P).transpose(0, 2, 1))
    yi = np.ascontiguousarray(yc.reshape(B, SEQ, CTX))

    in_maps = []
    for c in range(N_CORES):
        lo = c * BPC
        in_maps.append({
            "F": Fb, "G": Gb, "PW": PWb, "PB": PBb,
            "XI": xi[lo:lo + BPC], "YI": yi[lo:lo + BPC],
            "M01": m01[lo:lo + BPC],
        })

    res = run_bass_kernel_spmd(nc, in_maps, core_ids=list(range(N_CORES)))
    out = np.concatenate([res.results[c]["out_o"] for c in range(N_CORES)], axis=0)
    a = np.concatenate([res.results[c]["a_o"] for c in range(N_CORES)], axis=0)
    return (out.astype(np.float32), a.astype(np.float32))
